# revision 21
# baseline (speedup 1.0000x reference)
"""Trainium2 Bass kernel for fused QKV-projection + multi-head attention.

Problem: x[2,2048,1024] @ W_qkv[1024,3072] + b -> split q/k/v -> 16 heads of
dim 64 -> softmax(q k^T / 8) v -> [2,2048,1024].

Sharding (8 cores): data-parallel over batch (2) x tensor-parallel over head
groups (4 heads per core).  Each core computes a disjoint output slice
[2048, 256]; no collectives are needed.

v2 design (from the v1 trace: PE busy 183us/207us wall, ACT 139us; wall ~=
PE_work + head + tail, so the rewrite attacks PE work and PE saturation):
- Scores are K=128 zero-padded matmuls (kTz per head, other head's 64
  contraction rows memset to 0) instead of v1's row-tiled K=64 pairs: every
  attention MM now runs in the same 128x128 tiling mode, killing the
  ~75ns 64<->128 mode-transition tax (v1: scores avg 268ns, AV 274ns for
  N=512; same-shape proj MMs ran 238ns).
- q is processed in 512-wide chunks: AV accumulators shrink to 1 PSUM bank
  per head, freeing 2 banks for projection accumulators.  PSUM = S0,S1
  (scores, 2 banks each: both heads side by side) + Y0,Y1 (1 each) +
  P0,P1 (proj, 1 each) = exactly 8 banks.
- One EXP per (chunk, kb) covers both heads ([128,1024] f32 PSUM read);
  ACT chain = 128 x ~1075ns = 137.7us total, < PE work, so ACT is never
  the pacer as long as proj keeps the PE fed.
- The QKV projection (44.6us of PE work) is interleaved into the attention
  slots by a deadline-driven scheduler: only q/k-pair0-t0 + v0/v1 run up
  front; every other unit is emitted inside the 128 (pr, chunk, kb) slots
  before that slot's scores, subject to availability deadlines (v[j] by
  slot j, K0[t] by slot 4t-1, pair-1 units before pr=1).  In-order PE
  queue => proj MMs fill what would otherwise be exp-wait stalls.
- vv is padded to 128 weight cols (ones col at 64, zeros above) so AV
  weight loads trigger FWL; AV out partitions 0:65 hold [numerator | den].
- fp8 was evaluated and rejected: DoubleRow needs both operands fp8;
  e8+v8 AV gives rel_err 0.0226 > 2e-2 gate; fp8 qk proj gives 0.032.
- No on-device output transpose: kernel returns y in [head, 65, T] layout
  (row 64 = softmax denominator); the host divides and transposes.
"""

import sys

sys.path.insert(0, "/opt/trn_rl_repo")

import numpy as np

import concourse.bacc as bacc
import concourse.bass as bass
import concourse.mybir as mybir
import concourse.tile as tile
from concourse.bass import ts

P = 128
T = 2048
D = 1024
NH = 4          # heads per core
HD = 64         # head dim
TB = T // P     # 16 t-blocks
CB = D // P     # 8 c-blocks
F32 = mybir.dt.float32
F16 = mybir.dt.float16

_CACHED = {}


def build_bass(finalize=True):
    nc = bacc.Bacc()

    xT_d = nc.dram_tensor("xT", [D, T], F16, kind="ExternalInput")
    # w pre-tiled on host to [partition, slab]: 4 qk slabs of [cb, 128]
    # (q0,k0,q1,k1) then v [cb, 256]; every DMA is contiguous per partition.
    w_d = nc.dram_tensor("w", [P, 6 * D], F16, kind="ExternalInput")
    bqk_d = nc.dram_tensor("bqk", [P, 4], F32, kind="ExternalInput")
    bv_d = nc.dram_tensor("bv", [1, NH * HD], F32, kind="ExternalInput")
    # y[h, 0:64, t] = unnormalized numerator; y[h, 64, t] = denominator
    y_d = nc.dram_tensor("y", [NH, HD + 1, T], F32, kind="ExternalOutput")

    with tile.TileContext(nc) as tc:
        with (
            tc.tile_pool(name="persist", bufs=1) as persist,
            tc.tile_pool(name="ystage", bufs=3) as ystage,
            tc.tile_pool(name="epool", bufs=3) as epool,
            tc.tile_pool(name="ps_s", bufs=1, space="PSUM") as ps_s,
            tc.tile_pool(name="ps_y", bufs=1, space="PSUM") as ps_y,
            tc.tile_pool(name="ps_p", bufs=1, space="PSUM") as ps_p,
        ):
            # kTz[pr][s]: [d, t] fp16, head s of pair pr on contraction rows
            # 64s:64s+64, other 64 rows zero
            kTz = [
                [persist.tile([P, T], F16, name=f"kTz{pr}{s}") for s in range(2)]
                for pr in range(2)
            ]
            # qTt[pr]: [d, t] fp16, pair-packed rows (head0 0:64, head1 64:128)
            qTt = [persist.tile([P, T], F16, name=f"qTt{pr}") for pr in range(2)]
            # V' padded to 128 weight cols: [t-part, h, 128]; col 64 = ones,
            # cols 65:128 = zeros (junk rows in PSUM, never read)
            vv = [
                persist.tile([P, NH, P], F16, name=f"vv{tb}") for tb in range(TB)
            ]
            # All constant fills go to the otherwise-idle GpSimd engine so the
            # DVE queue is pure bias-adds and starts the moment proj PSUM
            # lands.  GpSimd wakes earliest (~5.8us) and does all of these by
            # ~9us; first consumer (scores kb=0) is at ~17us.
            bqk_sb = persist.tile([P, 4], F32)
            bvb = persist.tile([P, NH * HD], F32)
            wdum = persist.tile([P, P], F16, name="wdum")
            xdum = persist.tile([P, 512], F16, name="xdum")
            nc.gpsimd.memset(wdum[:], 0.0)
            nc.gpsimd.memset(xdum[:], 0.0)
            # bvb broadcast before the bulk memsets: v0's bias-add needs it
            # at ~20us; the memset consumers (scores, AV) come later
            nc.sync.dma_start(out=bqk_sb[:], in_=bqk_d[:, :])
            nc.gpsimd.dma_start(
                out=bvb[:], in_=bv_d[0:1, :].to_broadcast((P, NH * HD))
            )
            for pr in range(2):
                nc.gpsimd.memset(kTz[pr][0][HD:P, :], 0.0)
                nc.gpsimd.memset(kTz[pr][1][0:HD, :], 0.0)
            for tb in range(TB):
                nc.gpsimd.memset(vv[tb][:, :, HD : HD + 1], 1.0)
                nc.gpsimd.memset(vv[tb][:, :, HD + 1 : P], 0.0)
            wct = [
                persist.tile([P, CB, P], F16, name=f"wct{i}") for i in range(4)
            ]
            wv = persist.tile([P, CB, NH * HD], F16)
            xTs = [persist.tile([P, T], F16, name=f"xTs{cb}") for cb in range(CB)]

            def dma_w(i, split=1):
                # split-way halves: 1KB/partition descriptors, still efficient;
                # lets the first cb-chunk MMs gate on half the weight slab
                hb = CB // split
                for h in range(split):
                    nc.sync.dma_start(
                        out=wct[i][:, h * hb : (h + 1) * hb, :],
                        in_=w_d[
                            :, i * D + h * hb * P : i * D + (h + 1) * hb * P
                        ].rearrange("p (cb col) -> p cb col", cb=hb),
                    )

            def dma_x(tch):
                for cb in range(CB):
                    nc.sync.dma_start(
                        out=xTs[cb][:, ts(tch, 512)],
                        in_=xT_d[ts(cb, P), ts(tch, 512)],
                    )

            # DMA order = first-consumption order: pair-0 q/k weights + v
            # weights + first x chunk, then the rest t-ordered.
            dma_w(0, split=2)
            dma_x(0)
            dma_w(1, split=2)
            for h in range(2):
                nc.sync.dma_start(
                    out=wv[:, 4 * h : 4 * h + 4, :],
                    in_=w_d[:, 4 * D + h * D : 4 * D + (h + 1) * D].rearrange(
                        "p (cb col) -> p cb col", cb=4
                    ),
                )
            dma_x(1)
            dma_w(2)
            dma_w(3)
            dma_x(2)
            dma_x(3)

            # ---------------- projection units ------------------------------
            _ptag = [0]

            def _ptile(shape):
                t_ = ps_p.tile(shape, F32, tag=f"P{_ptag[0] % 2}", name="proj")
                _ptag[0] += 1
                return t_

            # PE warmup: dummy matmuls on zeroed scratch while the real input
            # DMAs are still in flight.  The tensor engine's clock ramps with
            # activity (~4us to full speed); this pays the ramp during the
            # DMA-gated head instead of on the first ~17 real matmuls.
            # Results land in the S0 bank, unused until the first scores.
            warm = ps_s.tile([P, 1024], F32, tag="S0", name="warm")
            for i in range(16):
                nc.tensor.matmul(
                    warm[:, ts(i % 2, 512)],
                    lhsT=wdum[:],
                    rhs=xdum[:],
                    start=True,
                    stop=True,
                )


            def qk_unit(ct, tc2):
                # one 512-wide t-chunk of q or k for pair ct//2
                pqk = _ptile([P, 512])
                for cb in range(CB):
                    nc.tensor.matmul(
                        pqk[:],
                        lhsT=wct[ct][:, cb, :],
                        rhs=xTs[cb][:, ts(tc2, 512)],
                        start=(cb == 0),
                        stop=(cb == CB - 1),
                    )
                # bias-adds run on ACT (Copy with per-partition bias AP): the
                # DVE is on the head's critical path, ACT idles there; Copy
                # shares Exp's table set so there is no table reload
                pr = ct // 2
                if ct % 2 == 0:  # q: pair-packed, one add
                    nc.scalar.activation(
                        out=qTt[pr][:, ts(tc2, 512)],
                        in_=pqk[:],
                        func=mybir.ActivationFunctionType.Identity,
                        bias=bqk_sb[:, ct : ct + 1],
                    )
                else:  # k: split per head into zero-padded kTz
                    for s in range(2):
                        nc.scalar.activation(
                            out=kTz[pr][s][ts(s, HD), ts(tc2, 512)],
                            in_=pqk[ts(s, HD), :],
                            func=mybir.ActivationFunctionType.Identity,
                            bias=bqk_sb[ts(s, HD), ct : ct + 1],
                        )

            def v_unit(tb, tag=None):
                pv = (
                    ps_s.tile([P, NH * HD], F32, tag=tag, name="proj")
                    if tag
                    else _ptile([P, NH * HD])
                )
                for cb in range(CB):
                    nc.tensor.matmul(
                        pv[:],
                        lhsT=xTs[cb][:, ts(tb, P)],
                        rhs=wv[:, cb, :],
                        start=(cb == 0),
                        stop=(cb == CB - 1),
                    )
                nc.vector.tensor_tensor(
                    out=vv[tb][:, :, 0:HD],
                    in0=pv[:].rearrange("p (a b) -> p a b", a=NH),
                    in1=bvb[:].rearrange("p (a b) -> p a b", a=NH),
                    op=mybir.AluOpType.add,
                )

            # ---------------- proj schedule ---------------------------------
            # Units not run up front, each with (emit_fn, deadline_slot).
            # Slots are global: idx = pr*64 + chunk*16 + kb, 0..127.
            pending = []
            for tb in range(2, TB):
                pending.append((lambda tb=tb: v_unit(tb), tb))
            for tcq in range(1, 4):
                pending.append((lambda t=tcq: qk_unit(1, t), 4 * tcq - 1))
            for tcq in range(1, 4):
                pending.append((lambda t=tcq: qk_unit(0, t), 16 * tcq - 1))
            for tcq in range(4):
                pending.append((lambda t=tcq: qk_unit(3, t), 64 + 4 * tcq - 1))
            for tcq in range(4):
                pending.append((lambda t=tcq: qk_unit(2, t), 64 + 16 * tcq - 1))
            pending.sort(key=lambda u: u[1])

            last_emit = [0]

            def emit_due(slot):
                # forced: deadline reached; voluntary: a steady drip spread
                # over the remaining slots so the PE always has filler work
                emitted = False
                while pending and pending[0][1] <= slot + 1:
                    pending.pop(0)[0]()
                    emitted = True
                if pending and not emitted:
                    spacing = max(2, (128 - slot) // (len(pending) + 1))
                    if slot - last_emit[0] >= spacing:
                        pending.pop(0)[0]()
                        emitted = True
                if emitted:
                    last_emit[0] = slot

            # head phase: minimum to start attention(pr=0, chunk=0)
            qk_unit(0, 0)
            qk_unit(1, 0)
            v_unit(0)
            v_unit(1)

            # ---------------- attention -------------------------------------
            def attention(pr):
                for chunk in range(4):
                    pY = [
                        ps_y.tile([P, 512], F32, tag=f"Y{s}", name=f"pY{s}")
                        for s in range(2)
                    ]

                    def issue_av(kb, eprev):
                        for s in range(2):
                            nc.tensor.matmul(
                                pY[s][:],
                                lhsT=vv[kb][:, 2 * pr + s, :],
                                rhs=eprev[:, ts(s, 512)],
                                start=(kb == 0),
                                stop=(kb == TB - 1),
                            )

                    prev = None
                    for kb in range(TB):
                        emit_due(pr * 64 + chunk * 16 + kb)
                        pS = ps_s.tile(
                            [P, 1024], F32, tag=f"S{kb % 2}", name="pS"
                        )
                        for s in range(2):
                            nc.tensor.matmul(
                                pS[:, ts(s, 512)],
                                lhsT=kTz[pr][s][:, ts(kb, P)],
                                rhs=qTt[pr][:, ts(chunk, 512)],
                                start=True,
                                stop=True,
                            )
                        eT = epool.tile(
                            [P, 1024], F16, tag=f"E{kb % 2}", name="eT"
                        )
                        nc.scalar.activation(
                            out=eT[:],
                            in_=pS[:],
                            func=mybir.ActivationFunctionType.Exp,
                            scale=0.125,
                        )
                        if prev is not None:
                            issue_av(kb - 1, prev)
                        prev = eT
                    issue_av(TB - 1, prev)
                    for s in range(2):
                        yst = ystage.tile([HD + 1, 512], F32, name="yst")
                        nc.vector.tensor_copy(out=yst[:], in_=pY[s][0 : HD + 1, :])
                        nc.sync.dma_start(
                            out=y_d[2 * pr + s, :, ts(chunk, 512)],
                            in_=yst[:],
                        )

            attention(0)
            attention(1)
            assert not pending, f"{len(pending)} proj units never emitted"

    if finalize:
        nc.finalize()
    return nc


def _shard_inputs(x, W_qkv, b_qkv):
    """Build per-core input maps. Core c: batch c//4, head group c%4."""
    x = np.asarray(x, dtype=np.float32)
    W = np.asarray(W_qkv, dtype=np.float32)
    b = np.asarray(b_qkv, dtype=np.float32)
    bf = np.float16
    xT = [np.ascontiguousarray(x[bi].T.astype(bf)) for bi in range(2)]
    in_maps = []
    for c in range(8):
        bi, hg = c // 4, c % 4
        cs = hg * 256  # column start within each of q/k/v blocks
        # per-pair packed q/k groups: [q_pair0 | k_pair0 | q_pair1 | k_pair1]
        cols = []
        bcols = []
        for pr in range(2):
            cols.append(W[:, cs + pr * 128 : cs + pr * 128 + 128])
            bcols.append(b[cs + pr * 128 : cs + pr * 128 + 128])
            cols.append(W[:, D + cs + pr * 128 : D + cs + pr * 128 + 128])
            bcols.append(b[D + cs + pr * 128 : D + cs + pr * 128 + 128])
        w_core = np.concatenate(
            [cols[0], cols[1], cols[2], cols[3], W[:, 2 * D + cs : 2 * D + cs + 256]],
            axis=1,
        ).astype(bf)
        w6 = w_core.reshape(8, 128, 768).transpose(1, 0, 2)  # [p, cb, col]
        w_core = np.concatenate(
            [w6[:, :, g * 128 : (g + 1) * 128].reshape(128, 1024) for g in range(4)]
            + [w6[:, :, 512:768].reshape(128, 2048)],
            axis=1,
        )  # [128, 6144]
        bqk = np.stack(bcols, axis=1)  # [128, 4]
        bv = np.ascontiguousarray(b[2 * D + cs : 2 * D + cs + 256].reshape(1, 256))
        in_maps.append(
            {
                "xT": xT[bi],
                "w": np.ascontiguousarray(w_core),
                "bqk": np.ascontiguousarray(bqk),
                "bv": bv,
            }
        )
    return in_maps


def _unshard_output(results):
    """results[c]["y"]: [4, 65, 2048] -> full [2, T, D] output."""
    out = np.empty((2, T, D), dtype=np.float32)
    for c in range(8):
        bi, hg = c // 4, c % 4
        yr = results[c]["y"]  # [NH, 65, T]
        y = yr[:, 0:HD, :] / yr[:, HD : HD + 1, :]  # [NH, HD, T]
        out[bi, :, hg * 256 : (hg + 1) * 256] = (
            y.transpose(2, 0, 1).reshape(T, NH * HD)
        )
    return out


def kernel(x, W_qkv, b_qkv, trace=False):
    from concourse.bass_utils import run_bass_kernel_spmd

    if "nc" not in _CACHED:
        _CACHED["nc"] = build_bass()
    nc = _CACHED["nc"]

    in_maps = _shard_inputs(x, W_qkv, b_qkv)
    res = run_bass_kernel_spmd(nc, in_maps, list(range(8)), trace=trace)
    _CACHED["last_result"] = res

    return _unshard_output(res.results)


if __name__ == "__main__":
    nc = build_bass()
    print("built ok")


# revision 23
# speedup vs baseline: 1.0279x; 1.0279x over previous
"""Trainium2 Bass kernel for fused QKV-projection + multi-head attention.

Problem: x[2,2048,1024] @ W_qkv[1024,3072] + b -> split q/k/v -> 16 heads of
dim 64 -> softmax(q k^T / 8) v -> [2,2048,1024].

Sharding (8 cores): data-parallel over batch (2) x tensor-parallel over head
groups (4 heads per core).  Each core computes a disjoint output slice
[2048, 256]; no collectives are needed.

v2 design (from the v1 trace: PE busy 183us/207us wall, ACT 139us; wall ~=
PE_work + head + tail, so the rewrite attacks PE work and PE saturation):
- Scores are K=128 zero-padded matmuls (kTz per head, other head's 64
  contraction rows memset to 0) instead of v1's row-tiled K=64 pairs: every
  attention MM now runs in the same 128x128 tiling mode, killing the
  ~75ns 64<->128 mode-transition tax (v1: scores avg 268ns, AV 274ns for
  N=512; same-shape proj MMs ran 238ns).
- q is processed in 512-wide chunks: AV accumulators shrink to 1 PSUM bank
  per head, freeing 2 banks for projection accumulators.  PSUM = S0,S1
  (scores, 2 banks each: both heads side by side) + Y0,Y1 (1 each) +
  P0,P1 (proj, 1 each) = exactly 8 banks.
- One EXP per (chunk, kb) covers both heads ([128,1024] f32 PSUM read);
  ACT chain = 128 x ~1075ns = 137.7us total, < PE work, so ACT is never
  the pacer as long as proj keeps the PE fed.
- The QKV projection (44.6us of PE work) is interleaved into the attention
  slots by a deadline-driven scheduler: only q/k-pair0-t0 + v0/v1 run up
  front; every other unit is emitted inside the 128 (pr, chunk, kb) slots
  before that slot's scores, subject to availability deadlines (v[j] by
  slot j, K0[t] by slot 4t-1, pair-1 units before pr=1).  In-order PE
  queue => proj MMs fill what would otherwise be exp-wait stalls.
- vv is padded to 128 weight cols (ones col at 64, zeros above) so AV
  weight loads trigger FWL; AV out partitions 0:65 hold [numerator | den].
- fp8 was evaluated and rejected: DoubleRow needs both operands fp8;
  e8+v8 AV gives rel_err 0.0226 > 2e-2 gate; fp8 qk proj gives 0.032.
- No on-device output transpose: kernel returns y in [head, 65, T] layout
  (row 64 = softmax denominator); the host divides and transposes.
"""

import sys

sys.path.insert(0, "/opt/trn_rl_repo")

import numpy as np

import concourse.bacc as bacc
import concourse.bass as bass
import concourse.mybir as mybir
import concourse.tile as tile
from concourse.bass import ts

P = 128
T = 2048
D = 1024
NH = 4          # heads per core
HD = 64         # head dim
TB = T // P     # 16 t-blocks
CB = D // P     # 8 c-blocks
F32 = mybir.dt.float32
F16 = mybir.dt.float16

_CACHED = {}


def build_bass(finalize=True):
    nc = bacc.Bacc()

    xT_d = nc.dram_tensor("xT", [D, T], F16, kind="ExternalInput")
    # w pre-tiled on host to [partition, slab]: 4 qk slabs of [cb, 128]
    # (q0,k0,q1,k1) then v [cb, 256]; every DMA is contiguous per partition.
    w_d = nc.dram_tensor("w", [P, 6 * D], F16, kind="ExternalInput")
    bqk_d = nc.dram_tensor("bqk", [P, 4], F32, kind="ExternalInput")
    bv_d = nc.dram_tensor("bv", [1, NH * HD], F32, kind="ExternalInput")
    # y[h, 0:64, t] = unnormalized numerator; y[h, 64, t] = denominator
    y_d = nc.dram_tensor("y", [NH, HD + 1, T], F32, kind="ExternalOutput")

    with tile.TileContext(nc) as tc:
        with (
            tc.tile_pool(name="persist", bufs=1) as persist,
            tc.tile_pool(name="ystage", bufs=3) as ystage,
            tc.tile_pool(name="epool", bufs=3) as epool,
            tc.tile_pool(name="ps_s", bufs=1, space="PSUM") as ps_s,
            tc.tile_pool(name="ps_y", bufs=1, space="PSUM") as ps_y,
            tc.tile_pool(name="ps_p", bufs=1, space="PSUM") as ps_p,
        ):
            # kTz[pr][s]: [d, t] fp16, head s of pair pr on contraction rows
            # 64s:64s+64, other 64 rows zero
            kTz = [
                [persist.tile([P, T], F16, name=f"kTz{pr}{s}") for s in range(2)]
                for pr in range(2)
            ]
            # qTt[pr]: [d, t] fp16, pair-packed rows (head0 0:64, head1 64:128)
            qTt = [persist.tile([P, T], F16, name=f"qTt{pr}") for pr in range(2)]
            # V' padded to 128 weight cols: [t-part, h, 128]; col 64 = ones,
            # cols 65:128 = zeros (junk rows in PSUM, never read)
            vv = [
                persist.tile([P, NH, P], F16, name=f"vv{tb}") for tb in range(TB)
            ]
            # All constant fills go to the otherwise-idle GpSimd engine so the
            # DVE queue is pure bias-adds and starts the moment proj PSUM
            # lands.  GpSimd wakes earliest (~5.8us) and does all of these by
            # ~9us; first consumer (scores kb=0) is at ~17us.
            bqk_sb = persist.tile([P, 4], F32)
            bvb = persist.tile([P, NH * HD], F32)
            wdum = persist.tile([P, P], F16, name="wdum")
            xdum = persist.tile([P, 512], F16, name="xdum")
            nc.gpsimd.memset(wdum[:], 0.0)
            nc.gpsimd.memset(xdum[:], 0.0)
            # bvb broadcast before the bulk memsets: v0's bias-add needs it
            # at ~20us; the memset consumers (scores, AV) come later
            nc.sync.dma_start(out=bqk_sb[:], in_=bqk_d[:, :])
            nc.gpsimd.dma_start(
                out=bvb[:], in_=bv_d[0:1, :].to_broadcast((P, NH * HD))
            )
            for pr in range(2):
                nc.gpsimd.memset(kTz[pr][0][HD:P, :], 0.0)
                nc.gpsimd.memset(kTz[pr][1][0:HD, :], 0.0)
            for tb in range(TB):
                nc.gpsimd.memset(vv[tb][:, :, HD : HD + 1], 1.0)
                nc.gpsimd.memset(vv[tb][:, :, HD + 1 : P], 0.0)
            wct = [
                persist.tile([P, CB, P], F16, name=f"wct{i}") for i in range(4)
            ]
            wv = persist.tile([P, CB, NH * HD], F16)
            xTs = [persist.tile([P, T], F16, name=f"xTs{cb}") for cb in range(CB)]

            def dma_w(i, split=1):
                # split-way halves: 1KB/partition descriptors, still efficient;
                # lets the first cb-chunk MMs gate on half the weight slab
                hb = CB // split
                for h in range(split):
                    nc.sync.dma_start(
                        out=wct[i][:, h * hb : (h + 1) * hb, :],
                        in_=w_d[
                            :, i * D + h * hb * P : i * D + (h + 1) * hb * P
                        ].rearrange("p (cb col) -> p cb col", cb=hb),
                    )

            def dma_x(tch):
                for cb in range(CB):
                    nc.sync.dma_start(
                        out=xTs[cb][:, ts(tch, 512)],
                        in_=xT_d[ts(cb, P), ts(tch, 512)],
                    )

            # DMA order = first-consumption order: pair-0 q/k weights + v
            # weights + first x chunk, then the rest t-ordered.
            dma_w(0, split=2)
            dma_x(0)
            dma_w(1, split=2)
            for h in range(2):
                nc.sync.dma_start(
                    out=wv[:, 4 * h : 4 * h + 4, :],
                    in_=w_d[:, 4 * D + h * D : 4 * D + (h + 1) * D].rearrange(
                        "p (cb col) -> p cb col", cb=4
                    ),
                )
            dma_x(1)
            dma_w(2)
            dma_w(3)
            dma_x(2)
            dma_x(3)

            # ---------------- projection units ------------------------------
            _ptag = [0]

            def _ptile(shape):
                t_ = ps_p.tile(shape, F32, tag=f"P{_ptag[0] % 2}", name="proj")
                _ptag[0] += 1
                return t_

            # PE warmup: dummy matmuls on zeroed scratch while the real input
            # DMAs are still in flight.  The tensor engine's clock ramps with
            # activity (~4us to full speed); this pays the ramp during the
            # DMA-gated head instead of on the first ~17 real matmuls.
            # Results land in the S0 bank, unused until the first scores.
            # 10 x ~427ns (half-clock) ends ~12.4us, right when the first
            # real unit's DMAs land: ramp covered, no handoff delay
            warm = ps_s.tile([P, 1024], F32, tag="S0", name="warm")
            for i in range(10):
                nc.tensor.matmul(
                    warm[:, ts(i % 2, 512)],
                    lhsT=wdum[:],
                    rhs=xdum[:],
                    start=True,
                    stop=True,
                )


            def qk_unit(ct, tc2):
                # one 512-wide t-chunk of q or k for pair ct//2
                pqk = _ptile([P, 512])
                for cb in range(CB):
                    nc.tensor.matmul(
                        pqk[:],
                        lhsT=wct[ct][:, cb, :],
                        rhs=xTs[cb][:, ts(tc2, 512)],
                        start=(cb == 0),
                        stop=(cb == CB - 1),
                    )
                # bias-adds run on ACT (Copy with per-partition bias AP): the
                # DVE is on the head's critical path, ACT idles there; Copy
                # shares Exp's table set so there is no table reload
                pr = ct // 2
                if ct % 2 == 0:  # q: pair-packed, one add
                    nc.vector.tensor_scalar_add(
                        out=qTt[pr][:, ts(tc2, 512)],
                        in0=pqk[:],
                        scalar1=bqk_sb[:, ct : ct + 1],
                    )
                else:  # k: split per head into zero-padded kTz
                    for s in range(2):
                        nc.vector.tensor_scalar_add(
                            out=kTz[pr][s][ts(s, HD), ts(tc2, 512)],
                            in0=pqk[ts(s, HD), :],
                            scalar1=bqk_sb[ts(s, HD), ct : ct + 1],
                        )

            def v_unit(tb, tag=None):
                pv = (
                    ps_s.tile([P, NH * HD], F32, tag=tag, name="proj")
                    if tag
                    else _ptile([P, NH * HD])
                )
                for cb in range(CB):
                    nc.tensor.matmul(
                        pv[:],
                        lhsT=xTs[cb][:, ts(tb, P)],
                        rhs=wv[:, cb, :],
                        start=(cb == 0),
                        stop=(cb == CB - 1),
                    )
                nc.vector.tensor_tensor(
                    out=vv[tb][:, :, 0:HD],
                    in0=pv[:].rearrange("p (a b) -> p a b", a=NH),
                    in1=bvb[:].rearrange("p (a b) -> p a b", a=NH),
                    op=mybir.AluOpType.add,
                )

            # ---------------- proj schedule ---------------------------------
            # Units not run up front, each with (emit_fn, deadline_slot).
            # Slots are global: idx = pr*64 + chunk*16 + kb, 0..127.
            pending = []
            for tb in range(2, TB):
                pending.append((lambda tb=tb: v_unit(tb), tb))
            for tcq in range(1, 4):
                pending.append((lambda t=tcq: qk_unit(1, t), 4 * tcq - 1))
            for tcq in range(1, 4):
                pending.append((lambda t=tcq: qk_unit(0, t), 16 * tcq - 1))
            for tcq in range(4):
                pending.append((lambda t=tcq: qk_unit(3, t), 64 + 4 * tcq - 1))
            for tcq in range(4):
                pending.append((lambda t=tcq: qk_unit(2, t), 64 + 16 * tcq - 1))
            pending.sort(key=lambda u: u[1])

            last_emit = [0]

            def emit_due(slot):
                # forced: deadline reached; voluntary: a steady drip spread
                # over the remaining slots so the PE always has filler work
                emitted = False
                while pending and pending[0][1] <= slot + 1:
                    pending.pop(0)[0]()
                    emitted = True
                if pending and not emitted:
                    spacing = max(2, (128 - slot) // (len(pending) + 1))
                    if slot - last_emit[0] >= spacing:
                        pending.pop(0)[0]()
                        emitted = True
                if emitted:
                    last_emit[0] = slot

            # head phase: minimum to start attention(pr=0, chunk=0)
            qk_unit(0, 0)
            qk_unit(1, 0)
            v_unit(0)
            v_unit(1)

            # ---------------- attention -------------------------------------
            def attention(pr):
                for chunk in range(4):
                    pY = [
                        ps_y.tile([P, 512], F32, tag=f"Y{s}", name=f"pY{s}")
                        for s in range(2)
                    ]

                    def issue_av(kb, eprev):
                        for s in range(2):
                            nc.tensor.matmul(
                                pY[s][:],
                                lhsT=vv[kb][:, 2 * pr + s, :],
                                rhs=eprev[:, ts(s, 512)],
                                start=(kb == 0),
                                stop=(kb == TB - 1),
                            )

                    prev = None
                    for kb in range(TB):
                        emit_due(pr * 64 + chunk * 16 + kb)
                        pS = ps_s.tile(
                            [P, 1024], F32, tag=f"S{kb % 2}", name="pS"
                        )
                        for s in range(2):
                            nc.tensor.matmul(
                                pS[:, ts(s, 512)],
                                lhsT=kTz[pr][s][:, ts(kb, P)],
                                rhs=qTt[pr][:, ts(chunk, 512)],
                                start=True,
                                stop=True,
                            )
                        eT = epool.tile(
                            [P, 1024], F16, tag=f"E{kb % 2}", name="eT"
                        )
                        nc.scalar.activation(
                            out=eT[:],
                            in_=pS[:],
                            func=mybir.ActivationFunctionType.Exp,
                            scale=0.125,
                        )
                        if prev is not None:
                            issue_av(kb - 1, prev)
                        prev = eT
                    issue_av(TB - 1, prev)
                    for s in range(2):
                        yst = ystage.tile([HD + 1, 512], F32, name="yst")
                        nc.vector.tensor_copy(out=yst[:], in_=pY[s][0 : HD + 1, :])
                        nc.sync.dma_start(
                            out=y_d[2 * pr + s, :, ts(chunk, 512)],
                            in_=yst[:],
                        )

            attention(0)
            attention(1)
            assert not pending, f"{len(pending)} proj units never emitted"

    if finalize:
        nc.finalize()
    return nc


def _shard_inputs(x, W_qkv, b_qkv):
    """Build per-core input maps. Core c: batch c//4, head group c%4."""
    x = np.asarray(x, dtype=np.float32)
    W = np.asarray(W_qkv, dtype=np.float32)
    b = np.asarray(b_qkv, dtype=np.float32)
    bf = np.float16
    xT = [np.ascontiguousarray(x[bi].T.astype(bf)) for bi in range(2)]
    in_maps = []
    for c in range(8):
        bi, hg = c // 4, c % 4
        cs = hg * 256  # column start within each of q/k/v blocks
        # per-pair packed q/k groups: [q_pair0 | k_pair0 | q_pair1 | k_pair1]
        cols = []
        bcols = []
        for pr in range(2):
            cols.append(W[:, cs + pr * 128 : cs + pr * 128 + 128])
            bcols.append(b[cs + pr * 128 : cs + pr * 128 + 128])
            cols.append(W[:, D + cs + pr * 128 : D + cs + pr * 128 + 128])
            bcols.append(b[D + cs + pr * 128 : D + cs + pr * 128 + 128])
        w_core = np.concatenate(
            [cols[0], cols[1], cols[2], cols[3], W[:, 2 * D + cs : 2 * D + cs + 256]],
            axis=1,
        ).astype(bf)
        w6 = w_core.reshape(8, 128, 768).transpose(1, 0, 2)  # [p, cb, col]
        w_core = np.concatenate(
            [w6[:, :, g * 128 : (g + 1) * 128].reshape(128, 1024) for g in range(4)]
            + [w6[:, :, 512:768].reshape(128, 2048)],
            axis=1,
        )  # [128, 6144]
        bqk = np.stack(bcols, axis=1)  # [128, 4]
        bv = np.ascontiguousarray(b[2 * D + cs : 2 * D + cs + 256].reshape(1, 256))
        in_maps.append(
            {
                "xT": xT[bi],
                "w": np.ascontiguousarray(w_core),
                "bqk": np.ascontiguousarray(bqk),
                "bv": bv,
            }
        )
    return in_maps


def _unshard_output(results):
    """results[c]["y"]: [4, 65, 2048] -> full [2, T, D] output."""
    out = np.empty((2, T, D), dtype=np.float32)
    for c in range(8):
        bi, hg = c // 4, c % 4
        yr = results[c]["y"]  # [NH, 65, T]
        y = yr[:, 0:HD, :] / yr[:, HD : HD + 1, :]  # [NH, HD, T]
        out[bi, :, hg * 256 : (hg + 1) * 256] = (
            y.transpose(2, 0, 1).reshape(T, NH * HD)
        )
    return out


def kernel(x, W_qkv, b_qkv, trace=False):
    from concourse.bass_utils import run_bass_kernel_spmd

    if "nc" not in _CACHED:
        _CACHED["nc"] = build_bass()
    nc = _CACHED["nc"]

    in_maps = _shard_inputs(x, W_qkv, b_qkv)
    res = run_bass_kernel_spmd(nc, in_maps, list(range(8)), trace=trace)
    _CACHED["last_result"] = res

    return _unshard_output(res.results)


if __name__ == "__main__":
    nc = build_bass()
    print("built ok")
